# revision 1
# baseline (speedup 1.0000x reference)
"""Single-head attention (B=4, N=2048, D=1024), scores scaled by 10.

Sharding: 8 cores = (batch, query-half). Core 2b+h owns queries
[1024h:1024(h+1)] of batch b. K/V projections are computed for the OWN
half only and exchanged with the pair core (2b ^ 1) via an on-chip
AllGather, halving the projection FLOPs vs recomputing the full
sequence per core. Key order is global [h0|h1] (rank order) on every
core, so the SPMD program is identical across cores.

Numerics: Q/K projections and Q@K^T run as bf16 hi/lo 3-pass matmuls
(error ~2^-17 — the x10 score scale amplifies rounding into the softmax
exponent); V projection and P@V run single-pass fp16. Scores are
k-partitioned (St tiles) so attention@V consumes P with no transposes;
per-query max is computed via fold + DVE 32x32 block transposes, and
max / 1-over-sum rows are broadcast across partitions with rank-1
matmuls.
"""

import numpy as np
import ml_dtypes

B, SEQ, D = 4, 2048, 1024
NQ = 1024          # queries per core (= keys computed per core)
QCH = 256          # attention q-chunk
NCH = NQ // QCH
NCORES = 8
DT = D // 128      # 8 d-tiles
ET = D // 128      # 8 e-tiles
KT = SEQ // 128    # 16 k-tiles
HKT = KT // 2      # 8 own-half k-tiles

_BUILT = {}


def _build():
    if "nc" in _BUILT:
        return _BUILT["nc"]
    from contextlib import ExitStack

    import concourse.bass as bass  # noqa: F401
    import concourse.mybir as mybir
    import concourse.tile as tile
    from concourse import bacc

    dt = mybir.dt
    F32, BF, F16 = dt.float32, dt.bfloat16, dt.float16
    AL = mybir.AluOpType
    EXP = mybir.ActivationFunctionType.Exp
    GROUPS = [[2 * i, 2 * i + 1] for i in range(NCORES // 2)]

    nc = bacc.Bacc("TRN2", target_bir_lowering=False, debug=False)

    xh_d = nc.dram_tensor("xh", [D, NQ], BF, kind="ExternalInput")
    xl_d = nc.dram_tensor("xl", [D, NQ], BF, kind="ExternalInput")
    wqh_d = nc.dram_tensor("wqh", [D, D], BF, kind="ExternalInput")
    wql_d = nc.dram_tensor("wql", [D, D], BF, kind="ExternalInput")
    wkh_d = nc.dram_tensor("wkh", [D, D], BF, kind="ExternalInput")
    wkl_d = nc.dram_tensor("wkl", [D, D], BF, kind="ExternalInput")
    wvf_d = nc.dram_tensor("wvf", [D, D], F16, kind="ExternalInput")
    ot_d = nc.dram_tensor("ot", [D, NQ], F32, kind="ExternalOutput")

    xh_r = xh_d.ap().rearrange("(t p) n -> p t n", p=128)
    xl_r = xl_d.ap().rearrange("(t p) n -> p t n", p=128)
    wqh_r = wqh_d.ap().rearrange("(t p) e -> p t e", p=128)
    wql_r = wql_d.ap().rearrange("(t p) e -> p t e", p=128)
    wkh_r = wkh_d.ap().rearrange("(t p) e -> p t e", p=128)
    wkl_r = wkl_d.ap().rearrange("(t p) e -> p t e", p=128)
    wvf_r = wvf_d.ap().rearrange("(t p) e -> p t e", p=128)
    ot_r = ot_d.ap().rearrange("(t p) q -> p t q", p=128)

    with tile.TileContext(nc) as tc, ExitStack() as ctx:
        qk_pool = ctx.enter_context(tc.tile_pool(name="qk", bufs=1))
        qth = qk_pool.tile([128, ET, NQ], BF, tag="qth")
        qtl = qk_pool.tile([128, ET, NQ], BF, tag="qtl")
        kth = qk_pool.tile([128, ET, SEQ], BF, tag="kth")
        ktl = qk_pool.tile([128, ET, SEQ], BF, tag="ktl")
        v_pool = ctx.enter_context(tc.tile_pool(name="vp", bufs=1))
        vf = v_pool.tile([128, KT, D], F16, tag="vf")

        const_pool = ctx.enter_context(tc.tile_pool(name="const", bufs=1))
        ones16 = const_pool.tile([128, 1], F16, tag="ones16")
        ten32 = const_pool.tile([1, 128], F32, tag="ten32")
        one32 = const_pool.tile([1, 128], F32, tag="one32")
        nc.vector.memset(ones16[:], 1.0)
        nc.vector.memset(ten32[:], 10.0)
        nc.vector.memset(one32[:], 1.0)

        dram = ctx.enter_context(tc.tile_pool(name="dram", bufs=1, space="DRAM"))
        # K hi rows 0:D, K lo rows D:2D (own half of k); AllGather output has
        # rank blocks [r0-hi, r0-lo, r1-hi, r1-lo]
        khl_in = dram.tile([2 * D, NQ], BF, tag="khl_in")
        khl_out = dram.tile([4 * D, NQ], BF, tag="khl_out")
        v_in = dram.tile([NQ, D], F16, tag="v_in")
        v_out = dram.tile([SEQ, D], F16, tag="v_out")
        warm_in = dram.tile([16, 16], BF, tag="warm_in")
        warm_out = dram.tile([32, 16], BF, tag="warm_out")

        # tiny warmup collective at t=0: pays the ncfw channel-setup latency
        # before the real exchanges need it
        warm_sb = const_pool.tile([16, 16], BF, tag="warm_sb")
        nc.vector.memset(warm_sb[:], 0.0)
        nc.sync.dma_start(warm_in[:], warm_sb[:])
        nc.gpsimd.collective_compute(
            "AllGather",
            AL.bypass,
            replica_groups=GROUPS,
            ins=[warm_in[:]],
            outs=[warm_out[:]],
        )

        # ---------------- Phase K: own-half K^T projection (bf16 3-pass) --
        with (
            tc.tile_pool(name="xspan", bufs=1) as xspan,
            tc.tile_pool(name="wstr", bufs=3) as wpool,
            tc.tile_pool(name="kev", bufs=3) as kevpool,
            tc.tile_pool(name="psA", bufs=4, space="PSUM") as psA,
        ):
            xh_t = xspan.tile([128, DT, NQ], BF, tag="xh")
            xl_t = xspan.tile([128, DT, NQ], BF, tag="xl")
            xf_t = xspan.tile([128, DT, NQ], F16, tag="xf")
            # first K weight tile before the x loads, and x split per d-tile,
            # so the first matmul starts after ~256KB instead of 4MB of DMA
            w0h_t = wpool.tile([128, DT, 128], BF, tag="wh")
            w0l_t = wpool.tile([128, DT, 128], BF, tag="wl")
            nc.sync.dma_start(w0h_t[:], wkh_r[:, :, 0:128])
            nc.sync.dma_start(w0l_t[:], wkl_r[:, :, 0:128])
            for dti in range(DT):
                nc.sync.dma_start(xh_t[:, dti, :], xh_r[:, dti, :])
                nc.sync.dma_start(xl_t[:, dti, :], xl_r[:, dti, :])
            for dti in range(DT):
                nc.vector.tensor_add(
                    xf_t[:, dti, :], xh_t[:, dti, :], xl_t[:, dti, :]
                )
            for et in range(ET):
                e0 = 128 * et
                if et == 0:
                    wh_t, wl_t = w0h_t, w0l_t
                else:
                    wh_t = wpool.tile([128, DT, 128], BF, tag="wh")
                    wl_t = wpool.tile([128, DT, 128], BF, tag="wl")
                    nc.sync.dma_start(wh_t[:], wkh_r[:, :, e0 : e0 + 128])
                    nc.sync.dma_start(wl_t[:], wkl_r[:, :, e0 : e0 + 128])
                for chn in range(NQ // 512):
                    n0 = 512 * chn
                    ps = psA.tile([128, 512], F32, tag="psA")
                    i = 0
                    for dti in range(DT):
                        for lw, rx in ((wh_t, xh_t), (wh_t, xl_t), (wl_t, xh_t)):
                            nc.tensor.matmul(
                                ps[:],
                                lw[:, dti, :],
                                rx[:, dti, n0 : n0 + 512],
                                start=(i == 0),
                                stop=(i == 3 * DT - 1),
                            )
                            i += 1
                    kh = kevpool.tile([128, 512], BF, tag="kevh")
                    kl = kevpool.tile([128, 512], BF, tag="kevl")
                    nc.vector.tensor_copy(kh[:], ps[:])
                    nc.vector.scalar_tensor_tensor(
                        kl[:], ps[:], 1.0, kh[:], op0=AL.mult, op1=AL.subtract
                    )
                    nc.sync.dma_start(
                        khl_in[e0 : e0 + 128, n0 : n0 + 512], kh[:]
                    )
                    nc.sync.dma_start(
                        khl_in[D + e0 : D + e0 + 128, n0 : n0 + 512], kl[:]
                    )

            # pair AllGather of K halves (readbacks are traced after phase Q
            # so their DMA-ring positions don't serialize the weight streams
            # behind the collective)
            nc.gpsimd.collective_compute(
                "AllGather",
                AL.bypass,
                replica_groups=GROUPS,
                ins=[khl_in[:]],
                outs=[khl_out[:]],
            )

            # ------------- Phase Q: own-half Q^T projection ---------------
            for et in range(ET):
                e0 = 128 * et
                wh_t = wpool.tile([128, DT, 128], BF, tag="wh")
                wl_t = wpool.tile([128, DT, 128], BF, tag="wl")
                nc.sync.dma_start(wh_t[:], wqh_r[:, :, e0 : e0 + 128])
                nc.sync.dma_start(wl_t[:], wql_r[:, :, e0 : e0 + 128])
                for chn in range(NQ // 512):
                    n0 = 512 * chn
                    ps = psA.tile([128, 512], F32, tag="psA")
                    i = 0
                    for dti in range(DT):
                        for lw, rx in ((wh_t, xh_t), (wh_t, xl_t), (wl_t, xh_t)):
                            nc.tensor.matmul(
                                ps[:],
                                lw[:, dti, :],
                                rx[:, dti, n0 : n0 + 512],
                                start=(i == 0),
                                stop=(i == 3 * DT - 1),
                            )
                            i += 1
                    hi = qth[:, et, n0 : n0 + 512]
                    nc.vector.tensor_copy(hi, ps[:])
                    nc.vector.scalar_tensor_tensor(
                        qtl[:, et, n0 : n0 + 512],
                        ps[:],
                        1.0,
                        hi,
                        op0=AL.mult,
                        op1=AL.subtract,
                    )

            # ------------- Phase V: own-half V projection (fp16) ----------
            with tc.tile_pool(name="wvp", bufs=1) as wvpool:
                for ec in range(2):
                    e0 = 512 * ec
                    wv_t = wvpool.tile([128, DT, 512], F16, tag="wv")
                    nc.sync.dma_start(wv_t[:], wvf_r[:, :, e0 : e0 + 512])
                    for kt in range(HKT):
                        k0 = 128 * kt
                        ps = psA.tile([128, 512], F32, tag="psA")
                        for dti in range(DT):
                            nc.tensor.matmul(
                                ps[:],
                                xf_t[:, dti, k0 : k0 + 128],
                                wv_t[:, dti, :],
                                start=(dti == 0),
                                stop=(dti == DT - 1),
                            )
                        vev = kevpool.tile([128, 512], F16, tag="vev")
                        nc.vector.tensor_copy(vev[:], ps[:])
                        nc.sync.dma_start(
                            v_in[k0 : k0 + 128, e0 : e0 + 512], vev[:]
                        )

            # ---- collective readbacks (K first — St needs it soonest) ----
            khl_out_r = khl_out[:].rearrange("(b t p) n -> b p t n", p=128, t=ET)
            for h in range(2):
                nc.gpsimd.dma_start(
                    kth[:, :, NQ * h : NQ * (h + 1)], khl_out_r[2 * h, :, :, :]
                )
                nc.gpsimd.dma_start(
                    ktl[:, :, NQ * h : NQ * (h + 1)], khl_out_r[2 * h + 1, :, :, :]
                )
            nc.gpsimd.collective_compute(
                "AllGather",
                AL.bypass,
                replica_groups=GROUPS,
                ins=[v_in[:]],
                outs=[v_out[:]],
            )
            v_out_r = v_out[:].rearrange("(b t p) e -> b p t e", p=128, t=HKT)
            for h in range(2):
                nc.gpsimd.dma_start(
                    vf[:, HKT * h : HKT * (h + 1), :], v_out_r[h, :, :, :]
                )

        # ---------------- Phase B: attention, q-chunked -------------------
        with (
            tc.tile_pool(name="stp", bufs=2) as stpool,
            tc.tile_pool(name="pp", bufs=2) as ppool,
            tc.tile_pool(name="tree", bufs=1) as treepool,
            tc.tile_pool(name="aux", bufs=2) as auxpool,
            tc.tile_pool(name="osb", bufs=3) as outpool,
            tc.tile_pool(name="psS", bufs=3, space="PSUM") as psS,
            tc.tile_pool(name="psO", bufs=2, space="PSUM") as psO,
            tc.tile_pool(name="psX", bufs=2, space="PSUM") as psX,
            tc.tile_pool(name="psR", bufs=1, space="PSUM") as psR,
        ):
            for c in range(NCH):
                q0 = QCH * c
                st = stpool.tile([128, KT, QCH], F32, tag="st")
                for kt in range(KT):
                    k0 = 128 * kt
                    ps = psS.tile([128, QCH], F32, tag="psS")
                    i = 0
                    for et in range(ET):
                        for lK, rQ in ((kth, qth), (kth, qtl), (ktl, qth)):
                            nc.tensor.matmul(
                                ps[:],
                                lK[:, et, k0 : k0 + 128],
                                rQ[:, et, q0 : q0 + QCH],
                                start=(i == 0),
                                stop=(i == 3 * ET - 1),
                            )
                            i += 1
                    nc.vector.tensor_copy(st[:, kt, :], ps[:])

                # ---- per-query max over all keys (k lives on partitions) --
                t8 = treepool.tile([128, 8, QCH], F32, tag="t8")
                for j in range(8):
                    nc.vector.tensor_max(
                        t8[:, j, :], st[:, 2 * j, :], st[:, 2 * j + 1, :]
                    )
                for j in range(4):
                    nc.vector.tensor_max(
                        t8[:, j, :], t8[:, 2 * j, :], t8[:, 2 * j + 1, :]
                    )
                nc.vector.tensor_max(t8[:, 0, :], t8[:, 0, :], t8[:, 1, :])
                nc.vector.tensor_max(t8[:, 2, :], t8[:, 2, :], t8[:, 3, :])
                nc.vector.tensor_max(t8[:, 0, :], t8[:, 0, :], t8[:, 2, :])
                # fold 128 partitions -> 32 (DVE ops need equal start
                # partitions, so move the 32-partition groups with DMAs)
                fold4 = treepool.tile([32, 4, QCH], F32, tag="fold4")
                for a in range(4):
                    nc.sync.dma_start(
                        fold4[:, a, :], t8[32 * a : 32 * (a + 1), 0, :]
                    )
                nc.vector.tensor_max(fold4[:, 0, :], fold4[:, 0, :], fold4[:, 1, :])
                nc.vector.tensor_max(fold4[:, 2, :], fold4[:, 2, :], fold4[:, 3, :])
                nc.vector.tensor_max(fold4[:, 0, :], fold4[:, 0, :], fold4[:, 2, :])
                t32t = treepool.tile([32, QCH], F32, tag="t32t")
                nc.vector.transpose(t32t[:], fold4[:, 0, :])
                # mx32[r, j] = max over partitions for query q0 + 32j + r
                mx32 = treepool.tile([32, 32], F32, tag="mx32")
                nc.vector.memset(mx32[:], 0.0)
                nc.vector.reduce_max(
                    mx32[:, 0 : QCH // 32],
                    t32t[:].rearrange("p (j c) -> p j c", c=32),
                    axis=mybir.AxisListType.X,
                )
                # transpose once more so q becomes (j-part, r-free) contiguous
                mx32t = treepool.tile([32, 32], F32, tag="mx32t")
                nc.vector.transpose(mx32t[:], mx32[:])
                m1row = treepool.tile([1, QCH], F32, tag="m1row")
                nc.sync.dma_start(m1row[:], mx32t[0 : QCH // 32, :])
                maxb_ps = psX.tile([128, QCH], F32, tag="bcast")
                nc.tensor.matmul(maxb_ps[:], ten32[:], m1row[:], start=True, stop=True)
                maxb = auxpool.tile([128, QCH], F32, tag="maxb")
                nc.vector.tensor_copy(maxb[:], maxb_ps[:])

                # ---- exp(10*s - 10*max) -> fp16 P ------------------------
                p_t = ppool.tile([128, KT, QCH], F16, tag="p")
                for kt in range(KT):
                    nc.vector.scalar_tensor_tensor(
                        st[:, kt, :],
                        st[:, kt, :],
                        10.0,
                        maxb[:],
                        op0=AL.mult,
                        op1=AL.subtract,
                    )
                    nc.scalar.activation(p_t[:, kt, :], st[:, kt, :], EXP)

                # ---- sums over keys via ones-matmul, then 1/sum ----------
                sum_ps = psR.tile([1, QCH], F32, tag="sum")
                for kt in range(KT):
                    nc.tensor.matmul(
                        sum_ps[:],
                        ones16[:],
                        p_t[:, kt, :],
                        start=(kt == 0),
                        stop=(kt == KT - 1),
                    )
                recrow = treepool.tile([1, QCH], F32, tag="recrow")
                nc.vector.reciprocal(recrow[:], sum_ps[:])
                recb_ps = psX.tile([128, QCH], F32, tag="bcast")
                nc.tensor.matmul(recb_ps[:], one32[:], recrow[:], start=True, stop=True)
                recb = auxpool.tile([128, QCH], F32, tag="recb")
                nc.vector.tensor_copy(recb[:], recb_ps[:])

                # ---- O^T[d, q] = V^T P, scaled by 1/sum ------------------
                for dti in range(DT):
                    d0 = 128 * dti
                    ops = psO.tile([128, QCH], F32, tag="psO")
                    for kt in range(KT):
                        nc.tensor.matmul(
                            ops[:],
                            vf[:, kt, d0 : d0 + 128],
                            p_t[:, kt, :],
                            start=(kt == 0),
                            stop=(kt == KT - 1),
                        )
                    osb = outpool.tile([128, QCH], F32, tag="osb")
                    nc.vector.scalar_tensor_tensor(
                        osb[:], ops[:], 1.0, recb[:], op0=AL.mult, op1=AL.mult
                    )
                    nc.sync.dma_start(ot_r[:, dti, q0 : q0 + QCH], osb[:])

    nc.compile()
    _BUILT["nc"] = nc
    return nc


def _prep_inputs(x, q_w, k_w, v_w):
    bf = ml_dtypes.bfloat16

    def hl(a):
        h = a.astype(bf)
        l_ = (a - h.astype(np.float32)).astype(bf)
        return h, l_

    wqh, wql = hl(np.ascontiguousarray(q_w.T))
    wkh, wkl = hl(np.ascontiguousarray(k_w.T))
    wvf = np.ascontiguousarray(v_w.T).astype(np.float16)

    in_maps = []
    for core in range(NCORES):
        b, h = divmod(core, 2)
        xt = np.ascontiguousarray(np.asarray(x[b, NQ * h : NQ * (h + 1)]).T)
        xh, xl = hl(xt)
        in_maps.append(
            {
                "xh": xh,
                "xl": xl,
                "wqh": wqh,
                "wql": wql,
                "wkh": wkh,
                "wkl": wkl,
                "wvf": wvf,
            }
        )
    return in_maps


def run(x, q_w, k_w, v_w, trace=False):
    from concourse.bass_utils import run_bass_kernel_spmd

    nc = _build()
    in_maps = _prep_inputs(x, q_w, k_w, v_w)
    res = run_bass_kernel_spmd(nc, in_maps, list(range(NCORES)), trace=trace)
    out = np.empty((B, SEQ, D), np.float32)
    for core in range(NCORES):
        b, h = divmod(core, 2)
        out[b, NQ * h : NQ * (h + 1)] = res.results[core]["ot"].T
    return out, res


def kernel(x, q_w, k_w, v_w):
    x = np.asarray(x, np.float32)
    q_w = np.asarray(q_w, np.float32)
    k_w = np.asarray(k_w, np.float32)
    v_w = np.asarray(v_w, np.float32)
    out, _ = run(x, q_w, k_w, v_w, trace=False)
    return out



# revision 2
# speedup vs baseline: 1.7241x; 1.7241x over previous
"""Single-head attention (B=4, N=2048, D=1024), scores scaled by 10.

Sharding: 8 cores = (batch, query-half). Core 2b+h owns queries
[1024h:1024(h+1)] of batch b. K/V projections are computed for the OWN
half only and exchanged with the pair core (2b ^ 1) via an on-chip
AllGather, halving the projection FLOPs vs recomputing the full
sequence per core. Key order is global [h0|h1] (rank order) on every
core, so the SPMD program is identical across cores.

Numerics: everything runs single-pass fp16 (fp32 PSUM accumulate).
Score error ~1.3e-2 in the softmax exponent after the x10 scale;
measured end-to-end rel err ~6e-3 vs the fp32 reference, within the
2e-2 gate. The per-query max subtraction cancels exactly in the
softmax normalization, so max precision only guards overflow.
Scores are k-partitioned (St tiles) so attention@V consumes P with no
transposes; per-query max is computed via fold + DVE 32x32 block
transposes, and max / 1-over-sum rows are broadcast across partitions
with rank-1 matmuls.
"""

import numpy as np

B, SEQ, D = 4, 2048, 1024
NQ = 1024          # queries per core (= keys computed per core)
QCH = 256          # attention q-chunk
NCH = NQ // QCH
NCORES = 8
DT = D // 128      # 8 d-tiles
ET = D // 128      # 8 e-tiles
KT = SEQ // 128    # 16 k-tiles
HKT = KT // 2      # 8 own-half k-tiles

_BUILT = {}


def _build():
    if "nc" in _BUILT:
        return _BUILT["nc"]
    from contextlib import ExitStack

    import concourse.bass as bass  # noqa: F401
    import concourse.mybir as mybir
    import concourse.tile as tile
    from concourse import bacc

    dt = mybir.dt
    F32, F16 = dt.float32, dt.float16
    BF = dt.bfloat16
    AL = mybir.AluOpType
    EXP = mybir.ActivationFunctionType.Exp
    GROUPS = [[2 * i, 2 * i + 1] for i in range(NCORES // 2)]

    nc = bacc.Bacc("TRN2", target_bir_lowering=False, debug=False)

    xf_d = nc.dram_tensor("xf", [D, NQ], F16, kind="ExternalInput")
    wq_d = nc.dram_tensor("wq", [D, D], F16, kind="ExternalInput")
    wk_d = nc.dram_tensor("wk", [D, D], F16, kind="ExternalInput")
    wv_d = nc.dram_tensor("wv", [D, D], F16, kind="ExternalInput")
    ot_d = nc.dram_tensor("ot", [D, NQ], F32, kind="ExternalOutput")

    xf_r = xf_d.ap().rearrange("(t p) n -> p t n", p=128)
    wq_r = wq_d.ap().rearrange("(t p) e -> p t e", p=128)
    wk_r = wk_d.ap().rearrange("(t p) e -> p t e", p=128)
    wv_r = wv_d.ap().rearrange("(t p) e -> p t e", p=128)
    ot_r = ot_d.ap().rearrange("(t p) q -> p t q", p=128)

    with tile.TileContext(nc) as tc, ExitStack() as ctx:
        qk_pool = ctx.enter_context(tc.tile_pool(name="qk", bufs=1))
        qt = qk_pool.tile([128, ET, NQ], F16, tag="qt")
        kt = qk_pool.tile([128, ET, SEQ], F16, tag="kt")
        v_pool = ctx.enter_context(tc.tile_pool(name="vp", bufs=1))
        vf = v_pool.tile([128, KT, D], F16, tag="vf")

        const_pool = ctx.enter_context(tc.tile_pool(name="const", bufs=1))
        ones16 = const_pool.tile([128, 1], F16, tag="ones16")
        ten32 = const_pool.tile([1, 128], F32, tag="ten32")
        one32 = const_pool.tile([1, 128], F32, tag="one32")
        nc.vector.memset(ones16[:], 1.0)
        nc.vector.memset(ten32[:], 10.0)
        nc.vector.memset(one32[:], 1.0)

        dram = ctx.enter_context(tc.tile_pool(name="dram", bufs=1, space="DRAM"))
        # own-half K^T rows; AllGather output has rank blocks [r0, r1]
        k_in = dram.tile([D, NQ], F16, tag="k_in")
        k_out = dram.tile([2 * D, NQ], F16, tag="k_out")
        v_in = dram.tile([NQ, D], F16, tag="v_in")
        v_out = dram.tile([SEQ, D], F16, tag="v_out")
        warm_in = dram.tile([16, 16], BF, tag="warm_in")
        warm_out = dram.tile([32, 16], BF, tag="warm_out")

        # tiny warmup collective at t=0: pays the ncfw channel-setup latency
        # before the real exchanges need it
        warm_sb = const_pool.tile([16, 16], BF, tag="warm_sb")
        nc.vector.memset(warm_sb[:], 0.0)
        nc.sync.dma_start(warm_in[:], warm_sb[:])
        nc.gpsimd.collective_compute(
            "AllGather",
            AL.bypass,
            replica_groups=GROUPS,
            ins=[warm_in[:]],
            outs=[warm_out[:]],
        )

        # ---------------- Phase K: own-half K^T projection (fp16) ---------
        with (
            tc.tile_pool(name="xspan", bufs=1) as xspan,
            tc.tile_pool(name="wstr", bufs=3) as wpool,
            tc.tile_pool(name="kev", bufs=3) as kevpool,
            tc.tile_pool(name="psA", bufs=4, space="PSUM") as psA,
        ):
            xf_t = xspan.tile([128, DT, NQ], F16, tag="xf")
            # first K weight tile before the x loads, and x split per d-tile,
            # so the first matmul starts after ~128KB instead of 2MB of DMA
            w0_t = wpool.tile([128, DT, 128], F16, tag="wt")
            nc.sync.dma_start(w0_t[:], wk_r[:, :, 0:128])
            for dti in range(DT):
                nc.sync.dma_start(xf_t[:, dti, :], xf_r[:, dti, :])
            for et in range(ET):
                e0 = 128 * et
                if et == 0:
                    w_t = w0_t
                else:
                    w_t = wpool.tile([128, DT, 128], F16, tag="wt")
                    nc.sync.dma_start(w_t[:], wk_r[:, :, e0 : e0 + 128])
                for chn in range(NQ // 512):
                    n0 = 512 * chn
                    ps = psA.tile([128, 512], F32, tag="psA")
                    for dti in range(DT):
                        nc.tensor.matmul(
                            ps[:],
                            w_t[:, dti, :],
                            xf_t[:, dti, n0 : n0 + 512],
                            start=(dti == 0),
                            stop=(dti == DT - 1),
                        )
                    kev = kevpool.tile([128, 512], F16, tag="kev")
                    nc.vector.tensor_copy(kev[:], ps[:])
                    nc.sync.dma_start(
                        k_in[e0 : e0 + 128, n0 : n0 + 512], kev[:]
                    )

            # pair AllGather of K halves (readbacks are traced after phase Q
            # so their DMA-ring positions don't serialize the weight streams
            # behind the collective)
            nc.gpsimd.collective_compute(
                "AllGather",
                AL.bypass,
                replica_groups=GROUPS,
                ins=[k_in[:]],
                outs=[k_out[:]],
            )

            # ------------- Phase Q: own-half Q^T projection ---------------
            for et in range(ET):
                e0 = 128 * et
                w_t = wpool.tile([128, DT, 128], F16, tag="wt")
                nc.sync.dma_start(w_t[:], wq_r[:, :, e0 : e0 + 128])
                for chn in range(NQ // 512):
                    n0 = 512 * chn
                    ps = psA.tile([128, 512], F32, tag="psA")
                    for dti in range(DT):
                        nc.tensor.matmul(
                            ps[:],
                            w_t[:, dti, :],
                            xf_t[:, dti, n0 : n0 + 512],
                            start=(dti == 0),
                            stop=(dti == DT - 1),
                        )
                    nc.vector.tensor_copy(qt[:, et, n0 : n0 + 512], ps[:])

            # ------------- Phase V: own-half V projection (fp16) ----------
            with tc.tile_pool(name="wvp", bufs=1) as wvpool:
                for ec in range(2):
                    e0 = 512 * ec
                    wv_t = wvpool.tile([128, DT, 512], F16, tag="wv")
                    nc.sync.dma_start(wv_t[:], wv_r[:, :, e0 : e0 + 512])
                    for kti in range(HKT):
                        k0 = 128 * kti
                        ps = psA.tile([128, 512], F32, tag="psA")
                        for dti in range(DT):
                            nc.tensor.matmul(
                                ps[:],
                                xf_t[:, dti, k0 : k0 + 128],
                                wv_t[:, dti, :],
                                start=(dti == 0),
                                stop=(dti == DT - 1),
                            )
                        vev = kevpool.tile([128, 512], F16, tag="vev")
                        nc.vector.tensor_copy(vev[:], ps[:])
                        nc.sync.dma_start(
                            v_in[k0 : k0 + 128, e0 : e0 + 512], vev[:]
                        )

            # ---- collective readbacks (K first — St needs it soonest) ----
            k_out_r = k_out[:].rearrange("(b t p) n -> b p t n", p=128, t=ET)
            for h in range(2):
                nc.gpsimd.dma_start(
                    kt[:, :, NQ * h : NQ * (h + 1)], k_out_r[h, :, :, :]
                )
            nc.gpsimd.collective_compute(
                "AllGather",
                AL.bypass,
                replica_groups=GROUPS,
                ins=[v_in[:]],
                outs=[v_out[:]],
            )
            v_out_r = v_out[:].rearrange("(b t p) e -> b p t e", p=128, t=HKT)
            for h in range(2):
                nc.gpsimd.dma_start(
                    vf[:, HKT * h : HKT * (h + 1), :], v_out_r[h, :, :, :]
                )

        # ---------------- Phase B: attention, q-chunked -------------------
        with (
            tc.tile_pool(name="stp", bufs=2) as stpool,
            tc.tile_pool(name="pp", bufs=2) as ppool,
            tc.tile_pool(name="tree", bufs=1) as treepool,
            tc.tile_pool(name="aux", bufs=2) as auxpool,
            tc.tile_pool(name="osb", bufs=3) as outpool,
            tc.tile_pool(name="psS", bufs=3, space="PSUM") as psS,
            tc.tile_pool(name="psO", bufs=2, space="PSUM") as psO,
            tc.tile_pool(name="psX", bufs=2, space="PSUM") as psX,
            tc.tile_pool(name="psR", bufs=1, space="PSUM") as psR,
        ):
            for c in range(NCH):
                q0 = QCH * c
                st = stpool.tile([128, KT, QCH], F32, tag="st")
                for kti in range(KT):
                    k0 = 128 * kti
                    ps = psS.tile([128, QCH], F32, tag="psS")
                    for et in range(ET):
                        nc.tensor.matmul(
                            ps[:],
                            kt[:, et, k0 : k0 + 128],
                            qt[:, et, q0 : q0 + QCH],
                            start=(et == 0),
                            stop=(et == ET - 1),
                        )
                    nc.vector.tensor_copy(st[:, kti, :], ps[:])

                # ---- per-query max over all keys (k lives on partitions) --
                t8 = treepool.tile([128, 8, QCH], F32, tag="t8")
                for j in range(8):
                    nc.vector.tensor_max(
                        t8[:, j, :], st[:, 2 * j, :], st[:, 2 * j + 1, :]
                    )
                for j in range(4):
                    nc.vector.tensor_max(
                        t8[:, j, :], t8[:, 2 * j, :], t8[:, 2 * j + 1, :]
                    )
                nc.vector.tensor_max(t8[:, 0, :], t8[:, 0, :], t8[:, 1, :])
                nc.vector.tensor_max(t8[:, 2, :], t8[:, 2, :], t8[:, 3, :])
                nc.vector.tensor_max(t8[:, 0, :], t8[:, 0, :], t8[:, 2, :])
                # fold 128 partitions -> 32 (DVE ops need equal start
                # partitions, so move the 32-partition groups with DMAs)
                fold4 = treepool.tile([32, 4, QCH], F32, tag="fold4")
                for a in range(4):
                    nc.sync.dma_start(
                        fold4[:, a, :], t8[32 * a : 32 * (a + 1), 0, :]
                    )
                nc.vector.tensor_max(fold4[:, 0, :], fold4[:, 0, :], fold4[:, 1, :])
                nc.vector.tensor_max(fold4[:, 2, :], fold4[:, 2, :], fold4[:, 3, :])
                nc.vector.tensor_max(fold4[:, 0, :], fold4[:, 0, :], fold4[:, 2, :])
                t32t = treepool.tile([32, QCH], F32, tag="t32t")
                nc.vector.transpose(t32t[:], fold4[:, 0, :])
                # mx32[r, j] = max over partitions for query q0 + 32j + r
                mx32 = treepool.tile([32, 32], F32, tag="mx32")
                nc.vector.memset(mx32[:], 0.0)
                nc.vector.reduce_max(
                    mx32[:, 0 : QCH // 32],
                    t32t[:].rearrange("p (j c) -> p j c", c=32),
                    axis=mybir.AxisListType.X,
                )
                # transpose once more so q becomes (j-part, r-free) contiguous
                mx32t = treepool.tile([32, 32], F32, tag="mx32t")
                nc.vector.transpose(mx32t[:], mx32[:])
                m1row = treepool.tile([1, QCH], F32, tag="m1row")
                nc.sync.dma_start(m1row[:], mx32t[0 : QCH // 32, :])
                maxb_ps = psX.tile([128, QCH], F32, tag="bcast")
                nc.tensor.matmul(maxb_ps[:], ten32[:], m1row[:], start=True, stop=True)
                maxb = auxpool.tile([128, QCH], F32, tag="maxb")
                nc.vector.tensor_copy(maxb[:], maxb_ps[:])

                # ---- exp(10*s - 10*max) -> fp16 P ------------------------
                p_t = ppool.tile([128, KT, QCH], F16, tag="p")
                for kti in range(KT):
                    nc.vector.scalar_tensor_tensor(
                        st[:, kti, :],
                        st[:, kti, :],
                        10.0,
                        maxb[:],
                        op0=AL.mult,
                        op1=AL.subtract,
                    )
                    nc.scalar.activation(p_t[:, kti, :], st[:, kti, :], EXP)

                # ---- sums over keys via ones-matmul, then 1/sum ----------
                sum_ps = psR.tile([1, QCH], F32, tag="sum")
                for kti in range(KT):
                    nc.tensor.matmul(
                        sum_ps[:],
                        ones16[:],
                        p_t[:, kti, :],
                        start=(kti == 0),
                        stop=(kti == KT - 1),
                    )
                recrow = treepool.tile([1, QCH], F32, tag="recrow")
                nc.vector.reciprocal(recrow[:], sum_ps[:])
                recb_ps = psX.tile([128, QCH], F32, tag="bcast")
                nc.tensor.matmul(recb_ps[:], one32[:], recrow[:], start=True, stop=True)
                recb = auxpool.tile([128, QCH], F32, tag="recb")
                nc.vector.tensor_copy(recb[:], recb_ps[:])

                # ---- O^T[d, q] = V^T P, scaled by 1/sum ------------------
                for dti in range(DT):
                    d0 = 128 * dti
                    ops = psO.tile([128, QCH], F32, tag="psO")
                    for kti in range(KT):
                        nc.tensor.matmul(
                            ops[:],
                            vf[:, kti, d0 : d0 + 128],
                            p_t[:, kti, :],
                            start=(kti == 0),
                            stop=(kti == KT - 1),
                        )
                    osb = outpool.tile([128, QCH], F32, tag="osb")
                    nc.vector.scalar_tensor_tensor(
                        osb[:], ops[:], 1.0, recb[:], op0=AL.mult, op1=AL.mult
                    )
                    nc.sync.dma_start(ot_r[:, dti, q0 : q0 + QCH], osb[:])

    nc.compile()
    _BUILT["nc"] = nc
    return nc


def _prep_inputs(x, q_w, k_w, v_w):
    wq = np.ascontiguousarray(q_w.T).astype(np.float16)
    wk = np.ascontiguousarray(k_w.T).astype(np.float16)
    wv = np.ascontiguousarray(v_w.T).astype(np.float16)

    in_maps = []
    for core in range(NCORES):
        b, h = divmod(core, 2)
        xt = np.ascontiguousarray(
            np.asarray(x[b, NQ * h : NQ * (h + 1)]).T
        ).astype(np.float16)
        in_maps.append({"xf": xt, "wq": wq, "wk": wk, "wv": wv})
    return in_maps


def run(x, q_w, k_w, v_w, trace=False):
    from concourse.bass_utils import run_bass_kernel_spmd

    nc = _build()
    in_maps = _prep_inputs(x, q_w, k_w, v_w)
    res = run_bass_kernel_spmd(nc, in_maps, list(range(NCORES)), trace=trace)
    out = np.empty((B, SEQ, D), np.float32)
    for core in range(NCORES):
        b, h = divmod(core, 2)
        out[b, NQ * h : NQ * (h + 1)] = res.results[core]["ot"].T
    return out, res


def kernel(x, q_w, k_w, v_w):
    x = np.asarray(x, np.float32)
    q_w = np.asarray(q_w, np.float32)
    k_w = np.asarray(k_w, np.float32)
    v_w = np.asarray(v_w, np.float32)
    out, _ = run(x, q_w, k_w, v_w, trace=False)
    return out


# revision 4
# speedup vs baseline: 1.7631x; 1.0226x over previous
"""Single-head attention (B=4, N=2048, D=1024), scores scaled by 10.

Sharding: 8 cores = (batch, query-half). Core 2b+h owns queries
[1024h:1024(h+1)] of batch b. K/V projections are computed for the OWN
half only and exchanged with the pair core (2b ^ 1) via an on-chip
AllGather, halving the projection FLOPs vs recomputing the full
sequence per core. Key order is global [h0|h1] (rank order) on every
core, so the SPMD program is identical across cores.

Numerics: everything runs single-pass fp16 (fp32 PSUM accumulate).
Measured end-to-end rel err ~6e-3 vs the fp32 reference (2e-2 gate).
The per-query max subtraction cancels exactly in the softmax
normalization, so max precision only guards overflow.

Schedule: phase order K -> V -> Q so both AllGathers launch early; all
weight/x DMAs are issued at t=0 (ordered by first use); collective
readbacks ride the scalar engine's queue so the V collective is not
serialized behind them on gpsimd. Phase B runs a software pipeline:
QK(c+1) matmuls are emitted around chunk c's softmax so the PE never
waits on the DVE max-tree / ACT exp chain; the max tree itself is
interleaved with the PSUM->SBUF score copies.
"""

import numpy as np

B, SEQ, D = 4, 2048, 1024
NQ = 1024          # queries per core (= keys computed per core)
QCH = 256          # attention q-chunk
NCH = NQ // QCH
NCORES = 8
DT = D // 128      # 8 d-tiles
ET = D // 128      # 8 e-tiles
KT = SEQ // 128    # 16 k-tiles
HKT = KT // 2      # 8 own-half k-tiles

_BUILT = {}


def _build():
    if "nc" in _BUILT:
        return _BUILT["nc"]
    from contextlib import ExitStack

    import concourse.bass as bass  # noqa: F401
    import concourse.mybir as mybir
    import concourse.tile as tile
    from concourse import bacc

    dt = mybir.dt
    F32, F16 = dt.float32, dt.float16
    BF = dt.bfloat16
    AL = mybir.AluOpType
    EXP = mybir.ActivationFunctionType.Exp
    GROUPS = [[2 * i, 2 * i + 1] for i in range(NCORES // 2)]

    nc = bacc.Bacc("TRN2", target_bir_lowering=False, debug=False)

    xf_d = nc.dram_tensor("xf", [D, NQ], F16, kind="ExternalInput")
    wq_d = nc.dram_tensor("wq", [D, D], F16, kind="ExternalInput")
    wk_d = nc.dram_tensor("wk", [D, D], F16, kind="ExternalInput")
    wv_d = nc.dram_tensor("wv", [D, D], F16, kind="ExternalInput")
    ot_d = nc.dram_tensor("ot", [D, NQ], F16, kind="ExternalOutput")

    xf_r = xf_d.ap().rearrange("(t p) n -> p t n", p=128)
    wq_r = wq_d.ap().rearrange("(t p) e -> p t e", p=128)
    wk_r = wk_d.ap().rearrange("(t p) e -> p t e", p=128)
    wv_r = wv_d.ap().rearrange("(t p) e -> p t e", p=128)
    ot_r = ot_d.ap().rearrange("(t p) q -> p t q", p=128)

    with tile.TileContext(nc) as tc, ExitStack() as ctx:
        qk_pool = ctx.enter_context(tc.tile_pool(name="qk", bufs=1))
        qt = qk_pool.tile([128, ET, NQ], F16, tag="qt")
        kt = qk_pool.tile([128, ET, SEQ], F16, tag="kt")
        v_pool = ctx.enter_context(tc.tile_pool(name="vp", bufs=1))
        vf = v_pool.tile([128, KT, D], F16, tag="vf")

        const_pool = ctx.enter_context(tc.tile_pool(name="const", bufs=1))
        ones16 = const_pool.tile([128, 1], F16, tag="ones16")
        ten32 = const_pool.tile([1, 128], F32, tag="ten32")
        one32 = const_pool.tile([1, 128], F32, tag="one32")

        dram = ctx.enter_context(tc.tile_pool(name="dram", bufs=1, space="DRAM"))
        # own-half K^T rows; AllGather output has rank blocks [r0, r1]
        k_in = dram.tile([D, NQ], F16, tag="k_in")
        k_out = dram.tile([2 * D, NQ], F16, tag="k_out")
        v_in = dram.tile([NQ, D], F16, tag="v_in")
        v_out = dram.tile([SEQ, D], F16, tag="v_out")
        warm_in = dram.tile([16, 16], BF, tag="warm_in")
        warm_out = dram.tile([32, 16], BF, tag="warm_out")

        # tiny warmup collective at t=0: pays the ncfw channel-setup latency
        # before the real exchanges need it
        warm_sb = const_pool.tile([16, 16], BF, tag="warm_sb")
        nc.vector.memset(warm_sb[:], 0.0)
        nc.sync.dma_start(warm_in[:], warm_sb[:])
        nc.gpsimd.collective_compute(
            "AllGather",
            AL.bypass,
            replica_groups=GROUPS,
            ins=[warm_in[:]],
            outs=[warm_out[:]],
        )
        nc.vector.memset(ones16[:], 1.0)
        nc.vector.memset(ten32[:], 10.0)
        nc.vector.memset(one32[:], 1.0)

        with (
            tc.tile_pool(name="xspan", bufs=1) as xspan,
            tc.tile_pool(name="wstr", bufs=1) as wpool,
            tc.tile_pool(name="kev", bufs=3) as kevpool,
            tc.tile_pool(name="psA", bufs=4, space="PSUM") as psA,
        ):
            xf_t = xspan.tile([128, DT, NQ], F16, tag="xf")
            wk_t = wpool.tile([128, DT, D], F16, tag="wk")
            wv_t = wpool.tile([128, DT, D], F16, tag="wv")
            wq_t = wpool.tile([128, DT, D], F16, tag="wq")
            # all input DMAs up front, ordered by first use: phase K's first
            # column block, then the rest rolling ahead of compute
            nc.sync.dma_start(wk_t[:, :, 0:128], wk_r[:, :, 0:128])
            for dti in range(DT):
                nc.sync.dma_start(xf_t[:, dti, 0:512], xf_r[:, dti, 0:512])
            for et in range(1, ET):
                e0 = 128 * et
                nc.sync.dma_start(wk_t[:, :, e0 : e0 + 128], wk_r[:, :, e0 : e0 + 128])
            for dti in range(DT):
                nc.sync.dma_start(xf_t[:, dti, 512:NQ], xf_r[:, dti, 512:NQ])
            for ec in range(2):
                e0 = 512 * ec
                nc.sync.dma_start(wv_t[:, :, e0 : e0 + 512], wv_r[:, :, e0 : e0 + 512])
            for et in range(ET):
                e0 = 128 * et
                nc.sync.dma_start(wq_t[:, :, e0 : e0 + 128], wq_r[:, :, e0 : e0 + 128])

            # ---------------- Phase K: own-half K^T projection -------------
            for chn in range(NQ // 512):
                n0 = 512 * chn
                for et in range(ET):
                    e0 = 128 * et
                    ps = psA.tile([128, 512], F32, tag="psA")
                    for dti in range(DT):
                        nc.tensor.matmul(
                            ps[:],
                            wk_t[:, dti, e0 : e0 + 128],
                            xf_t[:, dti, n0 : n0 + 512],
                            start=(dti == 0),
                            stop=(dti == DT - 1),
                        )
                    kev = kevpool.tile([128, 512], F16, tag="kev")
                    nc.vector.tensor_copy(kev[:], ps[:])
                    nc.sync.dma_start(
                        k_in[e0 : e0 + 128, n0 : n0 + 512], kev[:]
                    )

            # pair AllGather of K halves
            nc.gpsimd.collective_compute(
                "AllGather",
                AL.bypass,
                replica_groups=GROUPS,
                ins=[k_in[:]],
                outs=[k_out[:]],
            )
            # K readback on the scalar engine's queue: it has nothing else
            # to do until phase B's exp, and this keeps the V collective
            # from queueing behind a 4MB DMA on gpsimd
            k_out_r = k_out[:].rearrange("(b t p) n -> b p t n", p=128, t=ET)
            for h in range(2):
                nc.scalar.dma_start(
                    kt[:, :, NQ * h : NQ * (h + 1)], k_out_r[h, :, :, :]
                )

            # ------------- Phase V: own-half V projection ------------------
            for ec in range(2):
                e0 = 512 * ec
                for kti in range(HKT):
                    k0 = 128 * kti
                    ps = psA.tile([128, 512], F32, tag="psA")
                    for dti in range(DT):
                        nc.tensor.matmul(
                            ps[:],
                            xf_t[:, dti, k0 : k0 + 128],
                            wv_t[:, dti, e0 : e0 + 512],
                            start=(dti == 0),
                            stop=(dti == DT - 1),
                        )
                    vev = kevpool.tile([128, 512], F16, tag="vev")
                    nc.vector.tensor_copy(vev[:], ps[:])
                    nc.sync.dma_start(
                        v_in[k0 : k0 + 128, e0 : e0 + 512], vev[:]
                    )

            nc.gpsimd.collective_compute(
                "AllGather",
                AL.bypass,
                replica_groups=GROUPS,
                ins=[v_in[:]],
                outs=[v_out[:]],
            )
            v_out_r = v_out[:].rearrange("(b t p) e -> b p t e", p=128, t=HKT)
            for h in range(2):
                nc.scalar.dma_start(
                    vf[:, HKT * h : HKT * (h + 1), :], v_out_r[h, :, :, :]
                )

            # ------------- Phase Q: own-half Q^T projection ----------------
            for et in range(ET):
                e0 = 128 * et
                for chn in range(NQ // 512):
                    n0 = 512 * chn
                    ps = psA.tile([128, 512], F32, tag="psA")
                    for dti in range(DT):
                        nc.tensor.matmul(
                            ps[:],
                            wq_t[:, dti, e0 : e0 + 128],
                            xf_t[:, dti, n0 : n0 + 512],
                            start=(dti == 0),
                            stop=(dti == DT - 1),
                        )
                    nc.vector.tensor_copy(qt[:, et, n0 : n0 + 512], ps[:])

        # ---------------- Phase B: attention, q-chunked, pipelined --------
        with (
            tc.tile_pool(name="stp", bufs=2) as stpool,
            tc.tile_pool(name="pp", bufs=2) as ppool,
            tc.tile_pool(name="tree", bufs=2) as treepool,
            tc.tile_pool(name="aux", bufs=2) as auxpool,
            tc.tile_pool(name="osb", bufs=3) as outpool,
            tc.tile_pool(name="psS", bufs=3, space="PSUM") as psS,
            tc.tile_pool(name="psO", bufs=2, space="PSUM") as psO,
            tc.tile_pool(name="psX", bufs=2, space="PSUM") as psX,
            tc.tile_pool(name="psR", bufs=1, space="PSUM") as psR,
        ):
            st_ap = {}
            t8_ap = {}
            p_ap = {}
            maxb_ap = {}
            m1_ap = {}

            def emit_qk_half(c, half):
                """Scores for k-tiles [8h, 8h+8) of chunk c, with the DVE
                max-tree levels interleaved behind the PSUM copies."""
                q0 = QCH * c
                if half == 0:
                    st_ap[c] = stpool.tile([128, KT, QCH], F32, tag="st", name="st")
                    t8_ap[c] = treepool.tile([128, 8, QCH], F32, tag="t8", name="t8")
                st, t8 = st_ap[c], t8_ap[c]
                for kti in range(8 * half, 8 * half + 8):
                    k0 = 128 * kti
                    ps = psS.tile([128, QCH], F32, tag="psS")
                    for et in range(ET):
                        nc.tensor.matmul(
                            ps[:],
                            kt[:, et, k0 : k0 + 128],
                            qt[:, et, q0 : q0 + QCH],
                            start=(et == 0),
                            stop=(et == ET - 1),
                        )
                    nc.vector.tensor_copy(st[:, kti, :], ps[:])
                    # interleave the max tree: level-0 after each odd tile,
                    # higher levels as their inputs complete
                    if kti % 2 == 1:
                        j = kti // 2
                        nc.vector.tensor_max(
                            t8[:, j, :], st[:, kti - 1, :], st[:, kti, :]
                        )
                        if j % 2 == 1:
                            nc.vector.tensor_max(
                                t8[:, j - 1, :], t8[:, j - 1, :], t8[:, j, :]
                            )
                        if j == 3:
                            nc.vector.tensor_max(
                                t8[:, 0, :], t8[:, 0, :], t8[:, 2, :]
                            )
                        if j == 7:
                            nc.vector.tensor_max(
                                t8[:, 4, :], t8[:, 4, :], t8[:, 6, :]
                            )

            def emit_tree_finish(c):
                """Final fold 128 partitions -> one [1, QCH] max row."""
                t8 = t8_ap[c]
                nc.vector.tensor_max(t8[:, 0, :], t8[:, 0, :], t8[:, 4, :])
                fold4 = treepool.tile([32, 4, QCH], F32, tag="fold4")
                for a in range(4):
                    nc.sync.dma_start(
                        fold4[:, a, :], t8[32 * a : 32 * (a + 1), 0, :]
                    )
                nc.vector.tensor_max(fold4[:, 0, :], fold4[:, 0, :], fold4[:, 1, :])
                nc.vector.tensor_max(fold4[:, 2, :], fold4[:, 2, :], fold4[:, 3, :])
                nc.vector.tensor_max(fold4[:, 0, :], fold4[:, 0, :], fold4[:, 2, :])
                t32t = treepool.tile([32, QCH], F32, tag="t32t")
                nc.vector.transpose(t32t[:], fold4[:, 0, :])
                mx32 = treepool.tile([32, 32], F32, tag="mx32")
                nc.vector.memset(mx32[:], 0.0)
                nc.vector.reduce_max(
                    mx32[:, 0 : QCH // 32],
                    t32t[:].rearrange("p (j c) -> p j c", c=32),
                    axis=mybir.AxisListType.X,
                )
                mx32t = treepool.tile([32, 32], F32, tag="mx32t")
                nc.vector.transpose(mx32t[:], mx32[:])
                m1row = treepool.tile([1, QCH], F32, tag="m1row")
                nc.sync.dma_start(m1row[:], mx32t[0 : QCH // 32, :])
                m1_ap[c] = m1row

            def emit_maxb(c):
                """Broadcast 10*max across partitions via rank-1 matmul."""
                maxb_ps = psX.tile([128, QCH], F32, tag="bcast")
                nc.tensor.matmul(
                    maxb_ps[:], ten32[:], m1_ap[c][:], start=True, stop=True
                )
                maxb = auxpool.tile([128, QCH], F32, tag="maxb")
                nc.vector.tensor_copy(maxb[:], maxb_ps[:])
                maxb_ap[c] = maxb

            def emit_stt_exp(c):
                """st = 10*st - maxb, then P = exp(st) in fp16 (batched)."""
                st, maxb = st_ap[c], maxb_ap[c]
                p_ap[c] = ppool.tile([128, KT, QCH], F16, tag="p", name="p")
                for kti in range(KT):
                    nc.vector.scalar_tensor_tensor(
                        st[:, kti, :],
                        st[:, kti, :],
                        10.0,
                        maxb[:],
                        op0=AL.mult,
                        op1=AL.subtract,
                    )
                    if kti % 4 == 3:
                        nc.scalar.activation(
                            p_ap[c][:, kti - 3 : kti + 1, :],
                            st[:, kti - 3 : kti + 1, :],
                            EXP,
                        )

            def emit_sum_recb(c):
                """Key-sums of P via ones-matmul, 1/sum, broadcast."""
                sum_ps = psR.tile([1, QCH], F32, tag="sum")
                for kti in range(KT):
                    nc.tensor.matmul(
                        sum_ps[:],
                        ones16[:],
                        p_ap[c][:, kti, :],
                        start=(kti == 0),
                        stop=(kti == KT - 1),
                    )
                recrow = treepool.tile([1, QCH], F32, tag="recrow")
                nc.vector.reciprocal(recrow[:], sum_ps[:])
                recb_ps = psX.tile([128, QCH], F32, tag="bcast")
                nc.tensor.matmul(
                    recb_ps[:], one32[:], recrow[:], start=True, stop=True
                )
                recb = auxpool.tile([128, QCH], F32, tag="recb")
                nc.vector.tensor_copy(recb[:], recb_ps[:])
                return recb

            def emit_pv(c, recb):
                """O^T[d, q] = V^T P scaled by 1/sum, written out as fp16."""
                q0 = QCH * c
                for dti in range(DT):
                    d0 = 128 * dti
                    ops = psO.tile([128, QCH], F32, tag="psO")
                    for kti in range(KT):
                        nc.tensor.matmul(
                            ops[:],
                            vf[:, kti, d0 : d0 + 128],
                            p_ap[c][:, kti, :],
                            start=(kti == 0),
                            stop=(kti == KT - 1),
                        )
                    osb = outpool.tile([128, QCH], F16, tag="osb")
                    nc.vector.scalar_tensor_tensor(
                        osb[:], ops[:], 1.0, recb[:], op0=AL.mult, op1=AL.mult
                    )
                    nc.sync.dma_start(ot_r[:, dti, q0 : q0 + QCH], osb[:])

            emit_qk_half(0, 0)
            emit_qk_half(0, 1)
            for c in range(NCH):
                if c + 1 < NCH:
                    emit_qk_half(c + 1, 0)
                emit_tree_finish(c)
                emit_maxb(c)
                emit_stt_exp(c)
                if c + 1 < NCH:
                    emit_qk_half(c + 1, 1)
                recb = emit_sum_recb(c)
                emit_pv(c, recb)

    nc.compile()
    _BUILT["nc"] = nc
    return nc


def _prep_inputs(x, q_w, k_w, v_w):
    wq = np.ascontiguousarray(q_w.T).astype(np.float16)
    wk = np.ascontiguousarray(k_w.T).astype(np.float16)
    wv = np.ascontiguousarray(v_w.T).astype(np.float16)

    in_maps = []
    for core in range(NCORES):
        b, h = divmod(core, 2)
        xt = np.ascontiguousarray(
            np.asarray(x[b, NQ * h : NQ * (h + 1)]).T
        ).astype(np.float16)
        in_maps.append({"xf": xt, "wq": wq, "wk": wk, "wv": wv})
    return in_maps


def run(x, q_w, k_w, v_w, trace=False):
    from concourse.bass_utils import run_bass_kernel_spmd

    nc = _build()
    in_maps = _prep_inputs(x, q_w, k_w, v_w)
    res = run_bass_kernel_spmd(nc, in_maps, list(range(NCORES)), trace=trace)
    out = np.empty((B, SEQ, D), np.float32)
    for core in range(NCORES):
        b, h = divmod(core, 2)
        out[b, NQ * h : NQ * (h + 1)] = res.results[core]["ot"].T.astype(np.float32)
    return out, res


def kernel(x, q_w, k_w, v_w):
    x = np.asarray(x, np.float32)
    q_w = np.asarray(q_w, np.float32)
    k_w = np.asarray(k_w, np.float32)
    v_w = np.asarray(v_w, np.float32)
    out, _ = run(x, q_w, k_w, v_w, trace=False)
    return out


# revision 6
# speedup vs baseline: 1.7780x; 1.0085x over previous
"""Single-head attention (B=4, N=2048, D=1024), scores scaled by 10.

Sharding: 8 cores = (batch, query-half). Core 2b+h owns queries
[1024h:1024(h+1)] of batch b. K/V projections are computed for the OWN
half only and exchanged with the pair core (2b ^ 1) via an on-chip
AllGather, halving the projection FLOPs vs recomputing the full
sequence per core. Key order is global [h0|h1] (rank order) on every
core, so the SPMD program is identical across cores.

Numerics: everything runs single-pass fp16 (fp32 PSUM accumulate).
Measured end-to-end rel err ~6e-3 vs the fp32 reference (2e-2 gate).
The per-query max subtraction cancels exactly in the softmax
normalization, so max precision only guards overflow.

Schedule: phase order K -> V -> Q so both AllGathers launch early; all
weight/x DMAs are issued at t=0 (ordered by first use); collective
readbacks ride the scalar engine's queue so the V collective is not
serialized behind them on gpsimd. Phase B runs a software pipeline:
QK(c+1) matmuls are emitted around chunk c's softmax so the PE never
waits on the DVE max-tree / ACT exp chain; the max tree itself is
interleaved with the PSUM->SBUF score copies.
"""

import numpy as np

B, SEQ, D = 4, 2048, 1024
NQ = 1024          # queries per core (= keys computed per core)
QCH = 256          # attention q-chunk
NCH = NQ // QCH
NCORES = 8
DT = D // 128      # 8 d-tiles
ET = D // 128      # 8 e-tiles
KT = SEQ // 128    # 16 k-tiles
HKT = KT // 2      # 8 own-half k-tiles

_BUILT = {}


def _build():
    if "nc" in _BUILT:
        return _BUILT["nc"]
    from contextlib import ExitStack

    import concourse.bass as bass  # noqa: F401
    import concourse.mybir as mybir
    import concourse.tile as tile
    from concourse import bacc

    dt = mybir.dt
    F32, F16 = dt.float32, dt.float16
    BF = dt.bfloat16
    AL = mybir.AluOpType
    EXP = mybir.ActivationFunctionType.Exp
    GROUPS = [[2 * i, 2 * i + 1] for i in range(NCORES // 2)]

    nc = bacc.Bacc("TRN2", target_bir_lowering=False, debug=False)

    # inputs are host-pre-tiled to [128, DT, cols] so every load is one
    # large DMA with 16KB-contiguous per-partition lines (~full HBM BW)
    xf_d = nc.dram_tensor("xf", [128, DT, NQ], F16, kind="ExternalInput")
    wq_d = nc.dram_tensor("wq", [128, DT, D], F16, kind="ExternalInput")
    wk_d = nc.dram_tensor("wk", [128, DT, D], F16, kind="ExternalInput")
    wv_d = nc.dram_tensor("wv", [128, DT, D], F16, kind="ExternalInput")
    ot_d = nc.dram_tensor("ot", [D, NQ], F16, kind="ExternalOutput")

    xf_r = xf_d.ap()
    wq_r = wq_d.ap()
    wk_r = wk_d.ap()
    wv_r = wv_d.ap()
    ot_r = ot_d.ap().rearrange("(t p) q -> p t q", p=128)

    with tile.TileContext(nc) as tc, ExitStack() as ctx:
        qk_pool = ctx.enter_context(tc.tile_pool(name="qk", bufs=1))
        qt = qk_pool.tile([128, ET, NQ], F16, tag="qt")
        kt = qk_pool.tile([128, ET, SEQ], F16, tag="kt")
        v_pool = ctx.enter_context(tc.tile_pool(name="vp", bufs=1))
        vf = v_pool.tile([128, KT, D], F16, tag="vf")

        const_pool = ctx.enter_context(tc.tile_pool(name="const", bufs=1))
        ones16 = const_pool.tile([128, 1], F16, tag="ones16")
        ten32 = const_pool.tile([1, 128], F32, tag="ten32")
        one32 = const_pool.tile([1, 128], F32, tag="one32")

        dram = ctx.enter_context(tc.tile_pool(name="dram", bufs=1, space="DRAM"))
        # own-half K^T rows; AllGather output has rank blocks [r0, r1]
        k_in = dram.tile([D, NQ], F16, tag="k_in")
        k_out = dram.tile([2 * D, NQ], F16, tag="k_out")
        v_in = dram.tile([NQ, D], F16, tag="v_in")
        v_out = dram.tile([SEQ, D], F16, tag="v_out")
        warm_in = dram.tile([16, 16], BF, tag="warm_in")
        warm_out = dram.tile([32, 16], BF, tag="warm_out")

        # tiny warmup collective at t=0: pays the ncfw channel-setup latency
        # before the real exchanges need it
        warm_sb = const_pool.tile([16, 16], BF, tag="warm_sb")
        nc.vector.memset(warm_sb[:], 0.0)
        nc.sync.dma_start(warm_in[:], warm_sb[:])
        nc.gpsimd.collective_compute(
            "AllGather",
            AL.bypass,
            replica_groups=GROUPS,
            ins=[warm_in[:]],
            outs=[warm_out[:]],
        )
        nc.vector.memset(ones16[:], 1.0)
        nc.vector.memset(ten32[:], 10.0)
        nc.vector.memset(one32[:], 1.0)

        with (
            tc.tile_pool(name="xspan", bufs=1) as xspan,
            tc.tile_pool(name="wstr", bufs=1) as wpool,
            tc.tile_pool(name="kev", bufs=3) as kevpool,
            tc.tile_pool(name="psA", bufs=4, space="PSUM") as psA,
        ):
            xf_t = xspan.tile([128, DT, NQ], F16, tag="xf")
            wk_t = wpool.tile([128, DT, D], F16, tag="wk")
            wv_t = wpool.tile([128, DT, D], F16, tag="wv")
            wq_t = wpool.tile([128, DT, D], F16, tag="wq")
            # all input DMAs up front, split across the two HWDGE rings
            # (sync -> qSPDynamicHW, scalar -> qActDynamicHW) and ordered by
            # first use; each is >=1MB with long contiguous lines
            nc.sync.dma_start(wk_t[:, :, 0:512], wk_r[:, :, 0:512])
            nc.sync.dma_start(xf_t[:, :, 0:512], xf_r[:, :, 0:512])
            nc.sync.dma_start(wk_t[:, :, 512:D], wk_r[:, :, 512:D])
            nc.sync.dma_start(xf_t[:, :, 512:NQ], xf_r[:, :, 512:NQ])
            nc.scalar.dma_start(wv_t[:], wv_r[:, :, :])
            nc.scalar.dma_start(wq_t[:], wq_r[:, :, :])

            # ---------------- Phase K: own-half K^T projection -------------
            for chn in range(NQ // 512):
                n0 = 512 * chn
                for et in range(ET):
                    e0 = 128 * et
                    ps = psA.tile([128, 512], F32, tag="psA")
                    for dti in range(DT):
                        nc.tensor.matmul(
                            ps[:],
                            wk_t[:, dti, e0 : e0 + 128],
                            xf_t[:, dti, n0 : n0 + 512],
                            start=(dti == 0),
                            stop=(dti == DT - 1),
                        )
                    kev = kevpool.tile([128, 512], F16, tag="kev")
                    nc.vector.tensor_copy(kev[:], ps[:])
                    nc.scalar.dma_start(
                        k_in[e0 : e0 + 128, n0 : n0 + 512], kev[:]
                    )

            # pair AllGather of K halves
            nc.gpsimd.collective_compute(
                "AllGather",
                AL.bypass,
                replica_groups=GROUPS,
                ins=[k_in[:]],
                outs=[k_out[:]],
            )
            # K readback on the scalar engine's queue: it has nothing else
            # to do until phase B's exp, and this keeps the V collective
            # from queueing behind a 4MB DMA on gpsimd
            k_out_r = k_out[:].rearrange("(b t p) n -> b p t n", p=128, t=ET)
            for h in range(2):
                nc.scalar.dma_start(
                    kt[:, :, NQ * h : NQ * (h + 1)], k_out_r[h, :, :, :]
                )

            # ------------- Phase V: own-half V projection ------------------
            for ec in range(2):
                e0 = 512 * ec
                for kti in range(HKT):
                    k0 = 128 * kti
                    ps = psA.tile([128, 512], F32, tag="psA")
                    for dti in range(DT):
                        nc.tensor.matmul(
                            ps[:],
                            xf_t[:, dti, k0 : k0 + 128],
                            wv_t[:, dti, e0 : e0 + 512],
                            start=(dti == 0),
                            stop=(dti == DT - 1),
                        )
                    vev = kevpool.tile([128, 512], F16, tag="vev")
                    nc.vector.tensor_copy(vev[:], ps[:])
                    nc.scalar.dma_start(
                        v_in[k0 : k0 + 128, e0 : e0 + 512], vev[:]
                    )

            nc.gpsimd.collective_compute(
                "AllGather",
                AL.bypass,
                replica_groups=GROUPS,
                ins=[v_in[:]],
                outs=[v_out[:]],
            )
            # vf readback on gpsimd: it naturally queues right behind the
            # V collective on that FIFO, leaving the scalar engine free for
            # phase B's exp chain
            v_out_r = v_out[:].rearrange("(b t p) e -> b p t e", p=128, t=HKT)
            for h in range(2):
                nc.gpsimd.dma_start(
                    vf[:, HKT * h : HKT * (h + 1), :], v_out_r[h, :, :, :]
                )

            # ------------- Phase Q: own-half Q^T projection ----------------
            for et in range(ET):
                e0 = 128 * et
                for chn in range(NQ // 512):
                    n0 = 512 * chn
                    ps = psA.tile([128, 512], F32, tag="psA")
                    for dti in range(DT):
                        nc.tensor.matmul(
                            ps[:],
                            wq_t[:, dti, e0 : e0 + 128],
                            xf_t[:, dti, n0 : n0 + 512],
                            start=(dti == 0),
                            stop=(dti == DT - 1),
                        )
                    nc.vector.tensor_copy(qt[:, et, n0 : n0 + 512], ps[:])

        # ---------------- Phase B: attention, q-chunked, pipelined --------
        with (
            tc.tile_pool(name="stp", bufs=2) as stpool,
            tc.tile_pool(name="pp", bufs=2) as ppool,
            tc.tile_pool(name="tree", bufs=2) as treepool,
            tc.tile_pool(name="aux", bufs=2) as auxpool,
            tc.tile_pool(name="osb", bufs=3) as outpool,
            tc.tile_pool(name="psS", bufs=3, space="PSUM") as psS,
            tc.tile_pool(name="psO", bufs=2, space="PSUM") as psO,
            tc.tile_pool(name="psX", bufs=2, space="PSUM") as psX,
            tc.tile_pool(name="psR", bufs=1, space="PSUM") as psR,
        ):
            st_ap = {}
            t8_ap = {}
            p_ap = {}
            maxb_ap = {}
            m1_ap = {}

            def emit_qk_half(c, half):
                """Scores for k-tiles [8h, 8h+8) of chunk c, with the DVE
                max-tree levels interleaved behind the PSUM copies."""
                q0 = QCH * c
                if half == 0:
                    st_ap[c] = stpool.tile([128, KT, QCH], F32, tag="st", name="st")
                    t8_ap[c] = treepool.tile([128, 8, QCH], F32, tag="t8", name="t8")
                st, t8 = st_ap[c], t8_ap[c]
                for kti in range(8 * half, 8 * half + 8):
                    k0 = 128 * kti
                    ps = psS.tile([128, QCH], F32, tag="psS")
                    for et in range(ET):
                        nc.tensor.matmul(
                            ps[:],
                            kt[:, et, k0 : k0 + 128],
                            qt[:, et, q0 : q0 + QCH],
                            start=(et == 0),
                            stop=(et == ET - 1),
                        )
                    nc.vector.tensor_copy(st[:, kti, :], ps[:])
                    # interleave the max tree: level-0 after each odd tile,
                    # higher levels as their inputs complete
                    if kti % 2 == 1:
                        j = kti // 2
                        nc.vector.tensor_max(
                            t8[:, j, :], st[:, kti - 1, :], st[:, kti, :]
                        )
                        if j % 2 == 1:
                            nc.vector.tensor_max(
                                t8[:, j - 1, :], t8[:, j - 1, :], t8[:, j, :]
                            )
                        if j == 3:
                            nc.vector.tensor_max(
                                t8[:, 0, :], t8[:, 0, :], t8[:, 2, :]
                            )
                        if j == 7:
                            nc.vector.tensor_max(
                                t8[:, 4, :], t8[:, 4, :], t8[:, 6, :]
                            )

            def emit_tree_finish(c):
                """Final fold 128 partitions -> one [1, QCH] max row."""
                t8 = t8_ap[c]
                nc.vector.tensor_max(t8[:, 0, :], t8[:, 0, :], t8[:, 4, :])
                fold4 = treepool.tile([32, 4, QCH], F32, tag="fold4")
                for a in range(4):
                    nc.sync.dma_start(
                        fold4[:, a, :], t8[32 * a : 32 * (a + 1), 0, :]
                    )
                nc.vector.tensor_max(fold4[:, 0, :], fold4[:, 0, :], fold4[:, 1, :])
                nc.vector.tensor_max(fold4[:, 2, :], fold4[:, 2, :], fold4[:, 3, :])
                nc.vector.tensor_max(fold4[:, 0, :], fold4[:, 0, :], fold4[:, 2, :])
                t32t = treepool.tile([32, QCH], F32, tag="t32t")
                nc.vector.transpose(t32t[:], fold4[:, 0, :])
                mx32 = treepool.tile([32, 32], F32, tag="mx32")
                nc.vector.memset(mx32[:], 0.0)
                nc.vector.reduce_max(
                    mx32[:, 0 : QCH // 32],
                    t32t[:].rearrange("p (j c) -> p j c", c=32),
                    axis=mybir.AxisListType.X,
                )
                mx32t = treepool.tile([32, 32], F32, tag="mx32t")
                nc.vector.transpose(mx32t[:], mx32[:])
                m1row = treepool.tile([1, QCH], F32, tag="m1row")
                nc.sync.dma_start(m1row[:], mx32t[0 : QCH // 32, :])
                m1_ap[c] = m1row

            def emit_maxb(c):
                """Broadcast 10*max across partitions via rank-1 matmul."""
                maxb_ps = psX.tile([128, QCH], F32, tag="bcast")
                nc.tensor.matmul(
                    maxb_ps[:], ten32[:], m1_ap[c][:], start=True, stop=True
                )
                maxb = auxpool.tile([128, QCH], F32, tag="maxb")
                nc.vector.tensor_copy(maxb[:], maxb_ps[:])
                maxb_ap[c] = maxb

            def emit_stt_exp(c):
                """st = 10*st - maxb, then P = exp(st) in fp16 (batched)."""
                st, maxb = st_ap[c], maxb_ap[c]
                p_ap[c] = ppool.tile([128, KT, QCH], F16, tag="p", name="p")
                for kti in range(KT):
                    nc.vector.scalar_tensor_tensor(
                        st[:, kti, :],
                        st[:, kti, :],
                        10.0,
                        maxb[:],
                        op0=AL.mult,
                        op1=AL.subtract,
                    )
                    if kti % 4 == 3:
                        nc.scalar.activation(
                            p_ap[c][:, kti - 3 : kti + 1, :],
                            st[:, kti - 3 : kti + 1, :],
                            EXP,
                        )

            def emit_sum_recb(c):
                """Key-sums of P via ones-matmul, 1/sum, broadcast."""
                sum_ps = psR.tile([1, QCH], F32, tag="sum")
                for kti in range(KT):
                    nc.tensor.matmul(
                        sum_ps[:],
                        ones16[:],
                        p_ap[c][:, kti, :],
                        start=(kti == 0),
                        stop=(kti == KT - 1),
                    )
                recrow = treepool.tile([1, QCH], F32, tag="recrow")
                nc.vector.reciprocal(recrow[:], sum_ps[:])
                recb_ps = psX.tile([128, QCH], F32, tag="bcast")
                nc.tensor.matmul(
                    recb_ps[:], one32[:], recrow[:], start=True, stop=True
                )
                recb = auxpool.tile([128, QCH], F32, tag="recb")
                nc.vector.tensor_copy(recb[:], recb_ps[:])
                return recb

            def emit_pv(c, recb):
                """O^T[d, q] = V^T P scaled by 1/sum, written out as fp16."""
                q0 = QCH * c
                for dti in range(DT):
                    d0 = 128 * dti
                    ops = psO.tile([128, QCH], F32, tag="psO")
                    for kti in range(KT):
                        nc.tensor.matmul(
                            ops[:],
                            vf[:, kti, d0 : d0 + 128],
                            p_ap[c][:, kti, :],
                            start=(kti == 0),
                            stop=(kti == KT - 1),
                        )
                    osb = outpool.tile([128, QCH], F16, tag="osb")
                    nc.vector.scalar_tensor_tensor(
                        osb[:], ops[:], 1.0, recb[:], op0=AL.mult, op1=AL.mult
                    )
                    nc.sync.dma_start(ot_r[:, dti, q0 : q0 + QCH], osb[:])

            emit_qk_half(0, 0)
            emit_qk_half(0, 1)
            for c in range(NCH):
                if c + 1 < NCH:
                    emit_qk_half(c + 1, 0)
                emit_tree_finish(c)
                emit_maxb(c)
                emit_stt_exp(c)
                if c + 1 < NCH:
                    emit_qk_half(c + 1, 1)
                recb = emit_sum_recb(c)
                emit_pv(c, recb)

    nc.compile()
    _BUILT["nc"] = nc
    return nc


def _tile_rows(a):
    """[D_in, cols] -> [128, DT, cols] so row d lands at [d % 128, d // 128]."""
    return np.ascontiguousarray(
        a.reshape(DT, 128, a.shape[1]).transpose(1, 0, 2)
    )


def _prep_inputs(x, q_w, k_w, v_w):
    wq = _tile_rows(q_w.T.astype(np.float16))
    wk = _tile_rows(k_w.T.astype(np.float16))
    wv = _tile_rows(v_w.T.astype(np.float16))

    in_maps = []
    for core in range(NCORES):
        b, h = divmod(core, 2)
        xt = _tile_rows(
            np.asarray(x[b, NQ * h : NQ * (h + 1)]).T.astype(np.float16)
        )
        in_maps.append({"xf": xt, "wq": wq, "wk": wk, "wv": wv})
    return in_maps


def run(x, q_w, k_w, v_w, trace=False):
    from concourse.bass_utils import run_bass_kernel_spmd

    nc = _build()
    in_maps = _prep_inputs(x, q_w, k_w, v_w)
    res = run_bass_kernel_spmd(nc, in_maps, list(range(NCORES)), trace=trace)
    out = np.empty((B, SEQ, D), np.float32)
    for core in range(NCORES):
        b, h = divmod(core, 2)
        out[b, NQ * h : NQ * (h + 1)] = res.results[core]["ot"].T.astype(np.float32)
    return out, res


def kernel(x, q_w, k_w, v_w):
    x = np.asarray(x, np.float32)
    q_w = np.asarray(q_w, np.float32)
    k_w = np.asarray(k_w, np.float32)
    v_w = np.asarray(v_w, np.float32)
    out, _ = run(x, q_w, k_w, v_w, trace=False)
    return out
